# revision 1
# baseline (speedup 1.0000x reference)
"""Trainium2 Bass kernel for nn_CAComm_54829552501030 (sparse_attention).

Math: the reference's attention collapses exactly. With
  s  = upsample2x(parent_x @ conv_kernel + conv_bias)
  Q  = leaf * Wq,  K = s * Wk,  V = s * Wv
  alpha = softmax(scores, axis=-1)                # rows sum to 1
  out[n, i] = sum_j alpha[n, i, j] * V[n, i, 0]   # V broadcasts over the
                                                  # *row* index i (TF bcast)
            = V[n, i, 0] * 1 = s[n, i] * Wv[0, 0]
so the output is exactly  upsample2x(parent_x @ (conv_kernel*Wv) + conv_bias*Wv),
independent of leaf_x / Wq / Wk (verified vs the jax reference, rel err ~1e-7).

Device work (pure data parallel over the 65536 parent pixels, 8 cores):
each core gets 8192 pixels packed as (128, 1024): partitions hold 8
independent pixel-groups x 16 channels; one 128x128 block-diagonal
stationary matmul (8 copies of the 16x16 conv matrix) computes all 8
groups at once; a per-partition bias add finishes the conv. The 2x2
nearest-neighbor upsample is pure duplication and is applied while
unsharding on the host.
"""

import sys

for _p in ("/opt/trn_rl_repo", "/opt/pypackages"):
    if _p not in sys.path:
        sys.path.append(_p)

import numpy as np

import concourse.bass as bass
import concourse.mybir as mybir
from concourse import bass_utils
from concourse.bass_utils import run_bass_kernel_spmd


def _ensure_trace_support():
    """run_bass_kernel_spmd(trace=True) — e.g. under BASS_TRACE=1 — needs
    antenv.axon_hooks, which this image lacks; register the equivalent
    ctypes NTFF hook so tracing works instead of crashing. Also make the
    post-trace artifact upload non-fatal when no bucket is reachable."""
    import types

    try:
        import antenv.axon_hooks  # noqa: F401
    except ImportError:
        hook = None
        try:
            from trn_agent_boot import trn_boot

            hook = trn_boot._ntff_profile_via_ctypes("/opt/axon/libaxon_pjrt.so")
        except Exception:
            pass
        mod = types.ModuleType("antenv.axon_hooks")
        mod.get_axon_ntff_profile_hook = lambda: hook
        sys.modules["antenv.axon_hooks"] = mod

    orig_upload = bass_utils.upload_artifacts
    if not getattr(orig_upload, "_safe", False):

        def _safe_upload(tmpdir):
            try:
                return orig_upload(tmpdir)
            except Exception:
                return tmpdir

        _safe_upload._safe = True
        bass_utils.upload_artifacts = _safe_upload


_ensure_trace_support()

N_CORES = 8
B, PH, PW, C = 4, 128, 128, 16       # parent_x shape
GROUPS = 128 // C                    # 8 channel-groups per partition dim
PIX_PER_CORE = B * PH * PW // N_CORES  # 8192
NFREE = PIX_PER_CORE // GROUPS       # 1024 pixels per group
F32 = mybir.dt.float32


def build_nc(warmup: int = 15) -> bass.Bass:
    """Pipeline per core (one HW DGE queue per issuing engine):
      sync   : DMA x[:, :512] in; then y[:, :512] and y[:, 512:] out
               (sequential on the warm queue; a second queue's activation
               is staggered ~1us, so parallel output is no faster)
      scalar : DMA wb (conv matrix + bias column), x[:, 512:] in
      tensor : `warmup` scratch bf16 matmuls (release the HAM clock gate
               during the DMA wait), then the two fp32 matmuls
      vector : bias adds fused into the PSUM->SBUF copies
    (only sync/scalar have HW DGE queues; gpsimd is slow SW DGE; tensor
    and vector cannot issue DMAs)
    """
    nc = bass.Bass()
    # x stays ONE (128,1024) param: the column-strided DRAM reads spread
    # across DRAM channels and are measurably faster than contiguous blocks
    x_ext = nc.declare_dram_parameter("x", [128, NFREE], F32, isOutput=False)
    wb_ext = nc.declare_dram_parameter("wb", [128, 129], F32, isOutput=False)
    y_ext = nc.declare_dram_parameter("y", [128, NFREE], F32, isOutput=True)

    BF16 = mybir.dt.bfloat16
    with (
        nc.sbuf_tensor("x_sb", [128, NFREE], F32) as x_sb,
        nc.sbuf_tensor("wb_sb", [128, 129], F32) as wb_sb,
        nc.sbuf_tensor("y_sb", [128, NFREE], F32) as y_sb,
        nc.sbuf_tensor("junk_sb", [128, 256], BF16) as junk_sb,
        nc.psum_tensor("ps0", [128, 512], F32) as ps0,
        nc.psum_tensor("ps1", [128, 512], F32) as ps1,
        nc.psum_tensor("ps_junk", [128, 256], F32) as ps_junk,
        nc.Block() as block,
        nc.semaphore("dsem0") as dsem0,
        nc.semaphore("dsem1") as dsem1,
        nc.semaphore("wsem") as wsem,
        nc.semaphore("msem") as msem,
        nc.semaphore("a0sem") as a0sem,
        nc.semaphore("a1sem") as a1sem,
        nc.semaphore("osem") as osem,
    ):
        w_ap = wb_sb[:, 0:128]
        b_ap = wb_sb[:, 128:129]

        @block.sync
        def _(sync):
            sync.dma_start(out=x_sb[:, 0:512], in_=x_ext[:, 0:512]).then_inc(dsem0, 16)
            sync.wait_ge(a0sem, 2)
            sync.dma_start(out=y_ext[:, 0:512], in_=y_sb[:, 0:512]).then_inc(osem, 16)
            sync.wait_ge(a1sem, 2)
            sync.dma_start(
                out=y_ext[:, 512:NFREE], in_=y_sb[:, 512:NFREE]
            ).then_inc(osem, 16)
            # no final completion waits: the framework's end-of-program
            # DRAIN on this engine already waits out its DGE queue

        @block.scalar
        def _(scalar):
            scalar.dma_start(out=wb_sb[:], in_=wb_ext[:]).then_inc(wsem, 16)
            scalar.dma_start(out=x_sb[:, 512:NFREE], in_=x_ext[:, 512:NFREE]).then_inc(
                dsem1, 16
            )

        @block.tensor
        def _(tensor):
            # Warm-up matmuls read junk_sb UNINITIALIZED: the values are
            # irrelevant (ps_junk is never read) and skipping the memset
            # dependency lets the PE busy-window start at engine release,
            # so the HAM clock-gate flips to 8/8 before the real matmuls.
            # (CoreSim is validated with warmup=0; it rejects the
            # uninitialized read that hardware doesn't care about.)
            for _ in range(warmup):
                tensor.matmul(
                    ps_junk[:], junk_sb[:, 0:128], junk_sb[:],
                    start=True, stop=True, skip_group_check=True,
                )
            tensor.wait_ge(wsem, 16)
            tensor.wait_ge(dsem0, 16)
            tensor.matmul(
                ps0[:], w_ap, x_sb[:, 0:512], start=True, stop=True
            ).then_inc(msem, 1)
            tensor.wait_ge(dsem1, 16)
            tensor.matmul(
                ps1[:], w_ap, x_sb[:, 512:NFREE], start=True, stop=True
            ).then_inc(msem, 1)

        @block.vector
        def _(vector):
            vector.wait_ge(wsem, 16)
            vector.wait_ge(msem, 1)
            vector.tensor_scalar_add(y_sb[:, 0:512], ps0[:], b_ap).then_inc(a0sem, 2)
            vector.wait_ge(msem, 2)
            vector.tensor_scalar_add(y_sb[:, 512:NFREE], ps1[:], b_ap).then_inc(
                a1sem, 2
            )

    return nc


_NC = None


def _get_nc() -> bass.Bass:
    global _NC
    if _NC is None:
        _NC = build_nc()
    return _NC


def _pack_inputs(parent_x, conv_kernel, conv_bias, Wv):
    wv = float(np.asarray(Wv).reshape(-1)[0])
    W = (np.asarray(conv_kernel, np.float32) * wv).astype(np.float32)   # (16,16)
    bias = (np.asarray(conv_bias, np.float32) * wv).astype(np.float32)  # (16,)

    # stationary: out = S.T @ rhs with S[16a+c, 16a+f] = W[c, f];
    # column 128 carries the per-partition bias
    WB = np.zeros((128, 129), np.float32)
    for a in range(GROUPS):
        WB[C * a : C * (a + 1), C * a : C * (a + 1)] = W
        WB[C * a : C * (a + 1), 128] = bias
    # x packed per core: row 16a+c = channel c of pixel-group a
    xf = np.ascontiguousarray(parent_x, dtype=np.float32).reshape(
        N_CORES, GROUPS, NFREE, C
    )
    xp = np.ascontiguousarray(xf.transpose(0, 1, 3, 2)).reshape(N_CORES, 128, NFREE)
    return xp, WB


def _unpack_output(y_shards):
    # y_shards: (8, 128, 1024) with row 16a+f = channel f of pixel-group a
    y = np.asarray(y_shards).reshape(N_CORES, GROUPS, C, NFREE)
    y = y.transpose(0, 1, 3, 2).reshape(B, PH, PW, C)
    out = np.broadcast_to(
        y[:, :, None, :, None, :], (B, PH, 2, PW, 2, C)
    ).reshape(B, 2 * PH, 2 * PW, C)
    return np.ascontiguousarray(out)


def kernel(parent_x, leaf_x, conv_kernel, conv_bias, Wq, Wk, Wv, **_unused):
    xp, WB = _pack_inputs(parent_x, conv_kernel, conv_bias, Wv)
    in_maps = [{"x": xp[k], "wb": WB} for k in range(N_CORES)]
    nc = _get_nc()
    res = run_bass_kernel_spmd(nc, in_maps, list(range(N_CORES))).results
    y = np.stack([res[k]["y"] for k in range(N_CORES)])
    return _unpack_output(y)


if __name__ == "__main__":
    rng = np.random.default_rng(0)
    inputs = {
        "parent_x": rng.standard_normal((B, PH, PW, C)).astype(np.float32),
        "leaf_x": rng.standard_normal((B, 2 * PH, 2 * PW, C)).astype(np.float32),
        "conv_kernel": (rng.standard_normal((C, C)) * 0.1).astype(np.float32),
        "conv_bias": (rng.standard_normal(C) * 0.1).astype(np.float32),
        "Wq": rng.standard_normal((1, C)).astype(np.float32),
        "Wk": rng.standard_normal((1, C)).astype(np.float32),
        "Wv": rng.standard_normal((1, 1)).astype(np.float32),
    }
    out = kernel(**inputs)
    wv = float(inputs["Wv"][0, 0])
    s = inputs["parent_x"] @ (inputs["conv_kernel"] * wv) + inputs["conv_bias"] * wv
    exp = np.repeat(np.repeat(s, 2, axis=1), 2, axis=2)
    rel = np.linalg.norm(out - exp) / np.linalg.norm(exp)
    print("self-check rel err:", rel)

